# revision 12
# baseline (speedup 1.0000x reference)
"""Trainium2 Bass kernel for AttnNoProjVal.

Per batch element b (one NeuronCore each, B=8), using the identity
  scores = q k^T = hs M hs^T + (hs u) 1^T + 1 (hs v)^T + bk.bq,
  M = Wk^T Wq (host-folded), u = Wk^T bq, v = Wq^T bk:
the v and constant terms are per-QUERY-column offsets, which cancel exactly
in softmax and are dropped; the u term is a per-KEY offset folded into the
exp bias. The kernel computes one fused projection g^T = M^T hs^T, then
  scoresT[kp,qp] = (g^T)[:,kp] . (hsq^T)[:,qp]
  E = exp(scoresT/32 + bias[kp])    bias = (hs u)/32 - 3 + mask (host-prep)
  out[qp,:] = (E^T [hs | 1]) / colsum -- colsum via an extra N=1 ones column.

vs the fp32r baseline: all matmul operands are fp16 (hardware matmul stride
215ns vs fp32r's 227ns for 512 moving rows -- the 187ns fp32r weight loads
don't fully hide behind streaming, 95ns fp16 loads do), and the ~10% of
masked-out key positions are compacted away on host (key dim 2048 -> 1920
padded), shrinking projection, scores, and attention-value matmuls by 1/16.
fp16 end-to-end error ~9e-4 (fp8/DoubleRow and truncated-SVD variants all
blow the 2e-2 budget: softmax needs ~1e-2 absolute logit precision).
"""

import sys

sys.path.insert(0, "/opt/trn_rl_repo")

from contextlib import ExitStack

import numpy as np

import concourse.tile as tile
from concourse import bacc, mybir
from concourse.bass_utils import run_bass_kernel_spmd

B, S, H = 8, 2048, 1024
N_CORES = 8
HC = H // 128   # 8 chunks of the hidden dim
QB = S // 512   # 4 query blocks
F32 = mybir.dt.float32
F16 = mybir.dt.float16

NKC_DEFAULT = 15  # key chunks after mask compaction (padded to 128)

_CACHED_NC = {}


def build_nc(nkc=NKC_DEFAULT):
    nk = nkc * 128
    nc = bacc.Bacc(None, target_bir_lowering=False)

    hstq = nc.dram_tensor("hstq", [H, S], F16, kind="ExternalInput")
    hstk = nc.dram_tensor("hstk", [H, nk], F16, kind="ExternalInput")
    hsbk = nc.dram_tensor("hsbk", [nk, H], F16, kind="ExternalInput")
    mt = nc.dram_tensor("mt", [H, H], F16, kind="ExternalInput")  # M = Wk^T Wq
    mk = nc.dram_tensor("mk", [nk], F32, kind="ExternalInput")  # exp bias per key
    out = nc.dram_tensor("out", [S, H], F32, kind="ExternalOutput")

    # key-block widths for the projection moving dim (<=512 each; narrow
    # first block so the first chain's DMA lands early)
    kb_ofs, kb_w = [0], [min(128, nk)]
    o = kb_w[0]
    while o < nk:
        w = min(512, nk - o)
        kb_ofs.append(o)
        kb_w.append(w)
        o += w

    with tile.TileContext(nc) as tc, ExitStack() as whole:
        singles = whole.enter_context(tc.tile_pool(name="singles", bufs=1))
        gt_pool = whole.enter_context(tc.tile_pool(name="gtp", bufs=1))
        hsbk_pool = whole.enter_context(tc.tile_pool(name="hsbkp", bufs=1))
        qcol_pool = whole.enter_context(tc.tile_pool(name="qcolp", bufs=2))

        junk = singles.tile([128, 512], F16, tag="junk", name="junk")
        nc.vector.memset(junk[:], 0.0)
        bias_sb = singles.tile([128, nkc], F32, tag="bias", name="bias_sb")
        ones_sb = singles.tile([128, 1], F16, tag="ones", name="ones_sb")
        nc.vector.memset(ones_sb[:], 1.0)

        # g^T = M^T hs^T over compacted keys; resident for the whole kernel
        gt = [gt_pool.tile([128, nk], F16, tag=f"gt{d}", name=f"gt{d}") for d in range(HC)]
        # one merged tile per input: each DMA trigger instruction costs
        # ~600ns on its issuing engine regardless of size, so issue FEW,
        # LARGE, multi-chunk DMAs spread across otherwise-idle engine queues
        hsbk_sb = hsbk_pool.tile([128, nkc, H], F16, tag="hsbk", name="hsbk_sb")
        hsbk_r = hsbk.ap().rearrange("(k p) h -> p k h", p=128)
        nc.scalar.dma_start(out=hsbk_sb[:, 0:8, :], in_=hsbk_r[:, 0:8, :])
        nc.scalar.dma_start(out=hsbk_sb[:, 8:nkc, :], in_=hsbk_r[:, 8:nkc, :])
        nc.scalar.dma_start(out=bias_sb[:], in_=mk.ap().rearrange("(j p) -> p j", p=128))
        hsbt = [hsbk_sb[:, k, :] for k in range(nkc)]

        # PE warm-up: keep the PE ticking through the initial DMA wait so the
        # HAM clock-gate opens before the first real matmul.
        with tc.tile_pool(name="psw", bufs=1, space="PSUM") as psw:
            pjunk = psw.tile([128, 512], F32, tag="pj", name="pj")
            for _ in range(30):
                nc.tensor.matmul(
                    pjunk[:], lhsT=junk[:, 0:128], rhs=junk[:], start=True, stop=True
                )

        # ---- Phase A: fused projection g^T into SBUF.
        with ExitStack() as pa:
            wt_pool = pa.enter_context(tc.tile_pool(name="wtp", bufs=1))
            psA = pa.enter_context(tc.tile_pool(name="psA", bufs=8, space="PSUM"))

            m_sb = wt_pool.tile([128, HC, H], F16, tag="m", name="m_sb")
            hstk_sb = wt_pool.tile([128, HC, nk], F16, tag="hstk", name="hstk_sb")
            m_r = mt.ap().rearrange("(c p) o -> p c o", p=128)
            hstk_r = hstk.ap().rearrange("(c p) k -> p c k", p=128)
            # start-critical, in need-order: first (narrow) key block + m
            # column 0, then m in column order just ahead of the oc chains
            nc.sync.dma_start(out=hstk_sb[:, :, 0:kb_w[0]], in_=hstk_r[:, :, 0:kb_w[0]])
            nc.gpsimd.dma_start(out=m_sb[:, :, 0:128], in_=m_r[:, :, 0:128])
            nc.gpsimd.dma_start(out=m_sb[:, :, 128:512], in_=m_r[:, :, 128:512])
            nc.gpsimd.dma_start(out=m_sb[:, :, 512:H], in_=m_r[:, :, 512:H])
            for kb in range(1, len(kb_ofs)):
                o, w = kb_ofs[kb], kb_w[kb]
                nc.sync.dma_start(out=hstk_sb[:, :, o:o + w], in_=hstk_r[:, :, o:o + w])
            q0 = qcol_pool.tile([128, HC, 512], F16, tag="qcol", name="qcol")
            nc.sync.dma_start(
                out=q0[:], in_=hstq.ap().rearrange("(c p) q -> p c q", p=128)[:, :, 0:512]
            )

            for kb in range(len(kb_ofs)):
                o, w = kb_ofs[kb], kb_w[kb]
                for oc in range(HC):
                    ps = psA.tile([128, 512], F32, tag="psA", name="psa")
                    for h in range(HC):
                        nc.tensor.matmul(
                            ps[:, 0:w],
                            lhsT=m_sb[:, h, oc * 128:(oc + 1) * 128],
                            rhs=hstk_sb[:, h, o:o + w],
                            start=(h == 0),
                            stop=(h == HC - 1),
                        )
                    nc.scalar.copy(out=gt[oc][:, o:o + w], in_=ps[:, 0:w])

        # ---- Phase B: scores^T -> exp -> attention-value, per 512-wide block
        # of query positions.
        with ExitStack() as pb:
            et_pool = pb.enter_context(tc.tile_pool(name="etp", bufs=1))
            ps_s = pb.enter_context(tc.tile_pool(name="pss", bufs=3, space="PSUM"))
            ps_o = pb.enter_context(tc.tile_pool(name="pso", bufs=2, space="PSUM"))
            ps_n = pb.enter_context(tc.tile_pool(name="psn", bufs=1, space="PSUM"))
            out_pool = pb.enter_context(tc.tile_pool(name="outp", bufs=2))
            r_pool = pb.enter_context(tc.tile_pool(name="rp", bufs=4))

            for qb in range(QB):
                if qb == 0:
                    qcol = q0
                else:
                    qcol = qnext
                if qb + 1 < QB:
                    # prefetch next query block while this one computes
                    qnext = qcol_pool.tile([128, HC, 512], F16, tag="qcol", name="qcol")
                    nc.sync.dma_start(
                        out=qnext[:],
                        in_=hstq.ap().rearrange("(c p) q -> p c q", p=128)[
                            :, :, (qb + 1) * 512:(qb + 2) * 512
                        ],
                    )
                et = [et_pool.tile([128, 512], F16, tag=f"et{k}", name=f"et{k}") for k in range(nkc)]
                for k in range(nkc):
                    ps = ps_s.tile([128, 512], F32, tag="pss", name="pss")
                    for d in range(HC):
                        nc.tensor.matmul(
                            ps[:],
                            lhsT=gt[d][:, k * 128:(k + 1) * 128],
                            rhs=qcol[:, d, :],
                            start=(d == 0),
                            stop=(d == HC - 1),
                        )
                    nc.scalar.activation(
                        out=et[k][:], in_=ps[:],
                        func=mybir.ActivationFunctionType.Exp,
                        scale=1.0 / 32.0,
                        bias=bias_sb[:, k:k + 1],
                    )
                for qs in range(4):
                    po0 = ps_o.tile([128, 512], F32, tag="po0", name="po0")
                    po1 = ps_o.tile([128, 512], F32, tag="po1", name="po1")
                    pn = ps_n.tile([128, 1], F32, tag="pn", name="pn")
                    for k in range(nkc):
                        lw = et[k][:, qs * 128:(qs + 1) * 128]
                        st, sp = (k == 0), (k == nkc - 1)
                        nc.tensor.matmul(po0[:], lhsT=lw, rhs=hsbt[k][:, 0:512], start=st, stop=sp)
                        nc.tensor.matmul(po1[:], lhsT=lw, rhs=hsbt[k][:, 512:1024], start=st, stop=sp)
                        nc.tensor.matmul(pn[:], lhsT=lw, rhs=ones_sb[:], start=st, stop=sp)
                    r = r_pool.tile([128, 1], F32, tag="r", name="r")
                    nc.vector.reciprocal(r[:], pn[:, 0:1])
                    ot = out_pool.tile([128, H], F32, tag="ot", name="ot")
                    row = qb * 512 + qs * 128
                    if qb == QB - 1 and qs == 3:
                        # last group: pipeline scale->store per half to shorten
                        # the serial tail
                        nc.vector.tensor_scalar_mul(out=ot[:, 0:512], in0=po0[:], scalar1=r[:])
                        nc.scalar.dma_start(out=out.ap()[row:row + 128, 0:512], in_=ot[:, 0:512])
                        nc.vector.tensor_scalar_mul(out=ot[:, 512:1024], in0=po1[:], scalar1=r[:])
                        nc.scalar.dma_start(out=out.ap()[row:row + 128, 512:1024], in_=ot[:, 512:1024])
                    else:
                        nc.vector.tensor_scalar_mul(out=ot[:, 0:512], in0=po0[:], scalar1=r[:])
                        nc.vector.tensor_scalar_mul(out=ot[:, 512:1024], in0=po1[:], scalar1=r[:])
                        nc.scalar.dma_start(out=out.ap()[row:row + 128, :], in_=ot[:])

    nc.finalize()
    return nc


def prep_inputs(hidden_states, key_padding_mask, Wq_w, Wq_b, Wk_w, Wk_b):
    """Host prep: fold weights, compact masked keys. Returns (nkc, in_maps)."""
    hs = np.ascontiguousarray(hidden_states, dtype=np.float32)
    mask = np.asarray(key_padding_mask, dtype=bool)
    wq = np.asarray(Wq_w, dtype=np.float64)
    wk = np.asarray(Wk_w, dtype=np.float64)
    bq = np.asarray(Wq_b, dtype=np.float64)
    m16 = np.ascontiguousarray((wk.T @ wq).astype(np.float32).astype(np.float16))
    u = (wk.T @ bq).astype(np.float32)                         # [h]
    hsu = hs.reshape(-1, H) @ u                                # [B*S]
    bias = (hsu.reshape(B, S) / 32.0 - 3.0).astype(np.float32)

    kmax = int((~mask).sum(axis=1).max())
    nkc = max(1, -(-kmax // 128))
    nk = nkc * 128

    in_maps = []
    for b in range(B):
        sel = np.flatnonzero(~mask[b])
        kk = len(sel)
        hs16 = hs[b].astype(np.float16)
        hstk = np.zeros((H, nk), np.float16)
        hstk[:, :kk] = hs16.T[:, sel]
        hsbk = np.zeros((nk, H), np.float16)
        hsbk[:kk] = hs16[sel]
        mkb = np.full(nk, -1e30, np.float32)
        mkb[:kk] = bias[b][sel]
        in_maps.append({
            "hstq": np.ascontiguousarray(hs16.T),
            "hstk": hstk,
            "hsbk": hsbk,
            "mt": m16,
            "mk": mkb,
        })
    return nkc, in_maps


def kernel(hidden_states, key_padding_mask, Wq_w, Wq_b, Wk_w, Wk_b):
    nkc, in_maps = prep_inputs(
        hidden_states, key_padding_mask, Wq_w, Wq_b, Wk_w, Wk_b
    )
    nc = _CACHED_NC.get(nkc)
    if nc is None:
        nc = _CACHED_NC[nkc] = build_nc(nkc)

    res = run_bass_kernel_spmd(nc, in_maps, core_ids=list(range(N_CORES)))
    return np.stack([res.results[b]["out"] for b in range(B)]).astype(np.float32)
